# revision 19
# baseline (speedup 1.0000x reference)
"""EnergyGCN Trainium2 kernel: 8-core SPMD Bass/Tile implementation.

Strategy (node sharding):
  - 50000 nodes sharded contiguously across 8 cores (6250 rows, 49 blocks of
    128 per core).  Hidden state h0 kept node-major in SBUF: h0[b] =
    [128 nodes, H].
  - Per layer l: per block: hb = relu(h0) (fp16), PE-transpose -> hT [H, d],
    hr = hT.T @ WrT (PSUM, node-major), scaled by dinv on the PSUM->SBUF
    copy: hsb[b] = dinv * hr (= h~, fp16, node-major).  hsb is DMA'd into
    one of two DRAM bounce halves (blocks nbB..nb-1 -> half A first, then
    0..nbB-1 -> half B) and each half AllGather'd into a [8*half, H] fp16
    table (Shared scratchpad output).
  - Edge messages h~[col] are fetched with batched gpsimd.dma_gather (int16
    row indices into the half tables, round-robin over 4 SWDGE queues) and
    aggregated on TensorE with one-hot segment matrices generated by DVE in
    batches of 8 chunks: S[e, d] = (dstrel_e == iota_d), so PSUM accumulates
        pa[d, h] = sum_e S[e,d] * msg[e,h] + sum_d' Is2[d',d] * hsb[d',h]
    with Is2 = diag(-2*deg_d + 1).  Epilogue: the -3*dinv_d dest scale rides
    the ACT PSUM->SBUF copy (per-partition scale), then DVE:
        h0 = gate_l * h0 + (-3*dinv) * pa
    which equals gate*h0 + 6*hr - 3*ahat(hr) with self-loops folded in.
  - relu applied on read (next layer / final lin2).

Reference math:
    h = relu(x @ W1 + b1); h0 = h
    for l: hr = h @ Wr[l].T ; hn = 6hr - 3*ahat(hr)
           h0 = (1+tanh(eps[l]))*h0 + hn ; h = relu(h0)
    out = h @ W2 + b2
with ahat(y) = segment_sum(w[:,None]*y[col], row, n), w = dinv[row]*dinv[col],
self-loops appended, deg = counts of row (incl self), dinv = rsqrt(deg).
"""

import math

import numpy as np

import concourse.bacc as bacc
import concourse.bass as bass
import concourse.mybir as mybir
import concourse.tile as tile
from concourse import bass_utils

F32 = mybir.dt.float32
F16 = mybir.dt.float16
I16 = mybir.dt.int16
AF = mybir.ActivationFunctionType
ALU = mybir.AluOpType

N_NODES = 50000
D_IN, D_H, D_OUT = 256, 128, 64
N_LAYERS = 4
N_CORES = 8


class Cfg:
    def __init__(self, n=N_NODES, n_cores=N_CORES, d_in=D_IN, d_h=D_H,
                 d_out=D_OUT, n_layers=N_LAYERS, sg_blocks=6, gather_bufs=3,
                 sgen_batch=8, n_queues=4, cmax=14, shared_tables=True,
                 direct_table=False):
        self.n = n
        self.n_cores = n_cores
        self.d_in = d_in
        self.d_h = d_h
        self.d_out = d_out
        self.n_layers = n_layers
        self.sg_blocks = sg_blocks
        self.gather_bufs = gather_bufs
        self.sgen_batch = sgen_batch
        self.n_queues = n_queues
        self.cmax = cmax
        self.shared_tables = shared_tables
        self.direct_table = direct_table
        assert n % n_cores == 0
        self.nsh = n // n_cores
        self.nb = (self.nsh + 127) // 128
        # half B = blocks [0, nbB), allgathered second; half A first
        self.nbB = (self.nb + 1) // 2
        self.half_b = 128 * self.nbB
        self.ord_blocks = list(range(self.nbB, self.nb)) + list(range(self.nbB))


class Plan:
    pass


def preprocess(edge_index: np.ndarray, cfg: Cfg) -> Plan:
    n, P, nsh, nb = cfg.n, cfg.n_cores, cfg.nsh, cfg.nb
    row = edge_index[0].astype(np.int64)
    col = edge_index[1].astype(np.int64)
    deg = np.bincount(row, minlength=n).astype(np.float64) + 1.0  # + self loop
    dinv = 1.0 / np.sqrt(deg)
    m3 = (-3.0 * dinv).astype(np.float32)           # -3*dinv (dest scale)
    is2 = (-2.0 * deg + 1.0).astype(np.float32)     # identity inject diag

    # source node -> (group, row in half table);  g=0: half A (off>=half_b)
    HB = cfg.half_b
    s = col // nsh
    off = col - s * nsh
    hA_pad = 128 * (nb - cfg.nbB)
    g_of = (off < HB).astype(np.int64)              # g=0 -> half A, g=1 -> B
    rowg = np.where(off >= HB, s * hA_pad + off - HB, s * HB + off)

    ordpos = np.zeros(nb, dtype=np.int64)           # block -> position
    for i, b in enumerate(cfg.ord_blocks):
        ordpos[b] = i
    SGB = cfg.sg_blocks

    core = row // nsh
    per_core_edges = []
    counts = np.zeros((P, nb, 2), dtype=np.int64)
    for r in range(P):
        m = core == r
        rr = row[m] - r * nsh
        cc = rowg[m]
        g = g_of[m]
        blk = rr // 128
        pos = ordpos[blk]
        order = np.lexsort((rr, pos, g, pos // SGB))
        per_core_edges.append((rr[order], cc[order], g[order], blk[order]))
        np.add.at(counts[r], (blk, g), 1)

    nchunks = (counts.max(axis=0) + 127) // 128     # [nb, 2]

    # chunk layout: supergroups of SGB blocks in ord_blocks order; per sg:
    # (g0 chunks for its blocks in order) then (g1 chunks).
    slot_off = {}
    sg_entries = []
    offc = 0
    for i0 in range(0, nb, SGB):
        blocks = cfg.ord_blocks[i0:i0 + SGB]
        entries = []
        for g in (0, 1):
            c0 = offc
            for b in blocks:
                slot_off[(b, g)] = offc
                offc += int(nchunks[b, g]) * 128
            entries.append((g, (offc - c0) // 128, c0 // 128))
        sg_entries.append((blocks, entries))
    total_slots = offc
    total_chunks = total_slots // 128

    per_core = []
    for r in range(P):
        rr, cc, g, blk = per_core_edges[r]
        idx = np.zeros(total_slots, dtype=np.int16)     # pad -> row 0 (valid)
        dstrel = np.full(total_slots, -1.0, dtype=np.float16)
        for b in range(nb):
            for gg in (0, 1):
                msel = (blk == b) & (g == gg)
                k = int(msel.sum())
                if k == 0:
                    continue
                o = slot_off[(b, gg)]
                idx[o:o + k] = cc[msel]
                dstrel[o:o + k] = (rr[msel] - b * 128).astype(np.float16)
        # dma_gather index wrapping: slot i -> [i % 16, i // 16]; the Q7
        # tx/rx cores read different partition groups -> replicate to 128.
        idx16 = np.ascontiguousarray(np.tile(idx.reshape(-1, 16).T, (8, 1)))
        dstrel128 = np.ascontiguousarray(
            dstrel.reshape(total_chunks, 128).T)        # [128, nchunk] fp16

        def colmat(v):
            out = np.zeros((nb * 128,), dtype=np.float64)
            out[:nsh] = v
            return np.ascontiguousarray(
                out.reshape(nb, 128).T.astype(np.float32))

        per_core.append(dict(
            idx16=idx16,
            dstrel=dstrel128,
            dinv_cols=colmat(dinv[r * nsh:(r + 1) * nsh]),
            m3_cols=colmat(m3[r * nsh:(r + 1) * nsh]),
            is2_cols=colmat(is2[r * nsh:(r + 1) * nsh]),
        ))

    plan = Plan()
    plan.cfg = cfg
    plan.nchunks = nchunks
    plan.sg_entries = sg_entries
    plan.total_chunks = total_chunks
    plan.total_slots = total_slots
    plan.per_core = per_core
    return plan


def build_bass(plan: Plan, gates):
    cfg = plan.cfg
    nsh, nb, P = cfg.nsh, cfg.nb, cfg.n_cores
    H, DI, DO, L = cfg.d_h, cfg.d_in, cfg.d_out, cfg.n_layers
    TC = plan.total_chunks
    TS16 = plan.total_slots // 16
    SB = cfg.sgen_batch
    hA = nsh - cfg.half_b
    hA_pad = 128 * (nb - cfg.nbB)
    rowsA, rowsB = P * hA_pad, P * cfg.half_b

    nc = bacc.Bacc("TRN2", target_bir_lowering=False, debug=False,
                   num_devices=P, num_swdge_queues=cfg.n_queues)

    KI = DI // 128
    xT = nc.dram_tensor("xT", [128, KI * nsh], F16, kind="ExternalInput")
    W1 = nc.dram_tensor("W1", [128, KI * H], F16, kind="ExternalInput")
    b1r = nc.dram_tensor("b1r", [1, H], F16, kind="ExternalInput")
    WrT = nc.dram_tensor("WrT", [128, L * H], F16, kind="ExternalInput")
    W2 = nc.dram_tensor("W2", [H, DO], F16, kind="ExternalInput")
    b2r = nc.dram_tensor("b2r", [1, DO], F16, kind="ExternalInput")
    idx16_d = nc.dram_tensor("idx16", [128, TS16], I16, kind="ExternalInput")
    dstrel_d = nc.dram_tensor("dstrel", [128, TC], F16, kind="ExternalInput")
    dinv_d = nc.dram_tensor("dinv_cols", [128, nb], F32, kind="ExternalInput")
    m3_d = nc.dram_tensor("m3_cols", [128, nb], F32, kind="ExternalInput")
    is2_d = nc.dram_tensor("is2_cols", [128, nb], F32, kind="ExternalInput")
    iota_d = nc.dram_tensor("iota_in", [128, SB * 128], F16,
                            kind="ExternalInput")
    ident_d = nc.dram_tensor("ident_in", [128, 128], F16, kind="ExternalInput")
    out_d = nc.dram_tensor("out", [nsh, DO], F32, kind="ExternalOutput")

    last_rows = nsh - (nb - 1) * 128

    def rows_of(b):
        return last_rows if b == nb - 1 else 128

    addr_space = "Shared" if cfg.shared_tables else "Local"

    with tile.TileContext(nc) as tc:
        with (
            tc.tile_pool(name="const", bufs=1) as cpool,
            tc.tile_pool(name="work", bufs=4) as work,
            tc.tile_pool(name="sgen", bufs=4) as sgen_pool,
            tc.tile_pool(name="ga", bufs=cfg.gather_bufs) as ga_pool,
            tc.tile_pool(name="gb", bufs=cfg.gather_bufs) as gb_pool,
            tc.tile_pool(name="pt", bufs=2, space="PSUM") as pt_pool,
            tc.tile_pool(name="ph", bufs=2, space="PSUM") as ph_pool,
            tc.tile_pool(name="pagg", bufs=3, space="PSUM") as pagg_pool,
            tc.tile_pool(name="dram", bufs=2, space="DRAM") as dram,
        ):
            # persistent per-block state (node-major)
            h0 = [cpool.tile([128, 128], F32, tag=f"h0_{b}", name=f"h0_{b}")
                  for b in range(nb)]
            hsb = [cpool.tile([128, 128], F16, tag=f"hs_{b}", name=f"hs_{b}")
                   for b in range(nb)]
            Is2t = [cpool.tile([128, 128], F16, tag=f"i2_{b}", name=f"i2_{b}")
                    for b in range(nb)]
            for b in range(nb):
                nc.vector.memset(h0[b][:], 0.0)

            idx_sb = cpool.tile([128, TS16], I16)
            nc.sync.dma_start(idx_sb[:], idx16_d[:, :])
            dstrel_sb = cpool.tile([128, TC], F16)
            nc.sync.dma_start(dstrel_sb[:], dstrel_d[:, :])
            dinv_sb = cpool.tile([128, nb], F32)
            nc.sync.dma_start(dinv_sb[:], dinv_d[:, :])
            m3_sb = cpool.tile([128, nb], F32)
            nc.sync.dma_start(m3_sb[:], m3_d[:, :])
            is2_sb = cpool.tile([128, nb], F32)
            nc.sync.dma_start(is2_sb[:], is2_d[:, :])

            W1_sb = cpool.tile([128, KI * H], F16)
            nc.sync.dma_start(W1_sb[:], W1[:, :])
            b1_sb = cpool.tile([1, H], F16)
            nc.sync.dma_start(b1_sb[:], b1r[:, :])
            WrT_sb = cpool.tile([128, L * H], F16)
            nc.sync.dma_start(WrT_sb[:], WrT[:, :])
            W2_sb = cpool.tile([H, DO], F16)
            nc.sync.dma_start(W2_sb[:], W2[:, :])
            b2_sb = cpool.tile([1, DO], F16)
            nc.sync.dma_start(b2_sb[:], b2r[:, :])
            ones_sb = cpool.tile([1, 128], F16)
            nc.vector.memset(ones_sb[:], 1.0)
            iota_sb = cpool.tile([128, SB * 128], F16)
            nc.sync.dma_start(iota_sb[:], iota_d[:, :])
            ident = cpool.tile([128, 128], F16)
            nc.sync.dma_start(ident[:], ident_d[:, :])
            for b in range(nb):
                nc.vector.tensor_scalar(Is2t[b][:], ident[:],
                                        is2_sb[:, b:b + 1], None, op0=ALU.mult)

            # ---- lin1: h0 = relu(x @ W1 + b1) (node-major) ----
            for b in range(nb):
                rows = rows_of(b)
                ps = ph_pool.tile([128, 128], F32, tag="ph")
                for k in range(KI):
                    xs = work.tile([128, 128], F16, tag="xs")
                    nc.sync.dma_start(
                        xs[:, :rows],
                        xT[:, k * nsh + b * 128:k * nsh + b * 128 + rows])
                    nc.tensor.matmul(ps[:rows, :], xs[:, :rows],
                                     W1_sb[:, k * H:(k + 1) * H],
                                     start=(k == 0), stop=False)
                nc.tensor.matmul(ps[:rows, :], ones_sb[:, :rows], b1_sb[:],
                                 start=False, stop=True)
                nc.scalar.activation(h0[b][:rows, :], ps[:rows, :], AF.Relu)

            # ---- layers ----
            qrr = [0]

            def next_q():
                q = qrr[0]
                qrr[0] = (q + 1) % cfg.n_queues
                return q

            if cfg.direct_table:
                pidv = nc.sync.partition_id()
                offA_base = nc.sync.compute_val(pidv * (hA_pad * H))
                offB_base = nc.sync.compute_val(pidv * (cfg.half_b * H))

            barr_z = {}
            for l in range(L):
                bounceA = dram.tile([hA_pad, H], F16, tag="bA")
                bounceB = dram.tile([cfg.half_b, H], F16, tag="bB")
                tableA = dram.tile([rowsA, H], F16, tag="tA",
                                   addr_space=addr_space)
                tableB = dram.tile([rowsB, H], F16, tag="tB",
                                   addr_space=addr_space)
                if cfg.direct_table:
                    flagA = dram.tile([1, H], F16, tag="fA")
                    flagB = dram.tile([1, H], F16, tag="fB")
                    flagoutA = dram.tile([P, H], F16, tag="foA")
                    flagoutB = dram.tile([P, H], F16, tag="foB")
                taap = tableA.opt()
                tbap = tableB.opt()
                for b in cfg.ord_blocks:
                    rows = rows_of(b)
                    hb = work.tile([128, 128], F16, tag="hb")
                    nc.scalar.activation(hb[:], h0[b][:], AF.Relu)
                    pt = pt_pool.tile([128, 128], F16, tag="pt")
                    nc.tensor.transpose(pt[:], hb[:], ident[:])
                    hT = work.tile([128, 128], F16, tag="hT")
                    nc.scalar.activation(hT[:], pt[:], AF.Copy)
                    ph = ph_pool.tile([128, 128], F32, tag="ph")
                    nc.tensor.matmul(ph[:], hT[:],
                                     WrT_sb[:, l * H:(l + 1) * H],
                                     start=True, stop=True)
                    nc.scalar.activation(hsb[b][:], ph[:], AF.Copy,
                                         scale=dinv_sb[:, b:b + 1])
                    in_half_a = b >= cfg.nbB
                    r0 = (b - cfg.nbB) * 128 if in_half_a else b * 128
                    bounce = bounceA if in_half_a else bounceB
                    nc.sync.dma_start(bounce[r0:r0 + rows, :],
                                      hsb[b][:rows, :])
                    last = (b == nb - 1) if in_half_a \
                        else (b == cfg.nbB - 1)
                    if last and cfg.direct_table:
                        g = 0 if in_half_a else 1
                        bnc, tap, base, flag, flagout, nrows = (
                            (bounceA, taap, offA_base, flagA, flagoutA, hA)
                            if in_half_a else
                            (bounceB, tbap, offB_base, flagB, flagoutB,
                             cfg.half_b))
                        dst = bass.AP(tap.tensor, base,
                                      [[H, nrows], [1, H]],
                                      dep_tracking_offset=0)
                        nc.sync.dma_start(dst, bnc.opt())
                        fsb = work.tile([1, H], F16, tag="flag")
                        nc.sync.dma_start(fsb[:], tap[0:1, :])
                        nc.sync.dma_start(flag[0:1, :], fsb[:])
                        nc.gpsimd.collective_compute(
                            "AllGather", ALU.bypass,
                            replica_groups=[list(range(P))],
                            ins=[flag.opt()], outs=[flagout.opt()])
                        bsb = work.tile([1, 16], F16, tag="barr",
                                        name=f"bsb{g}_{l}")
                        nc.sync.dma_start(bsb[:], flagout[0:1, 0:16])
                        breg = nc.gpsimd.alloc_register(f"barr_{g}_{l}")
                        nc.gpsimd.reg_load(
                            breg, bsb.bitcast(mybir.dt.uint32)[0:1, 0:1])
                        barr_z[g] = nc.gpsimd.compute_val(
                            nc.gpsimd.snap(breg, donate=True) * 0)
                    elif last:
                        bnc, tbl_t = ((bounceA, tableA) if in_half_a
                                      else (bounceB, tableB))
                        nc.gpsimd.collective_compute(
                            "AllGather", ALU.bypass,
                            replica_groups=[list(range(P))],
                            ins=[bnc.opt()], outs=[tbl_t.opt()])

                tbl = {0: tableA.opt(), 1: tableB.opt()}
                gp = {0: ga_pool, 1: gb_pool}
                for si, (blocks, entries) in enumerate(plan.sg_entries):
                    bufs = {}
                    base_chunk = {}
                    for (g, nch, ch0) in entries:
                        if nch == 0:
                            continue
                        mb = gp[g].tile([128, nch, H], F16, tag=f"g{g}",
                                        name=f"mb{g}_{si}")
                        for t0 in range(0, nch, cfg.cmax):
                            tn = min(cfg.cmax, nch - t0)
                            c0 = ch0 + t0
                            nreg = (barr_z[g] + tn * 128) \
                                if cfg.direct_table else tn * 128
                            nc.gpsimd.dma_gather(
                                mb[:, t0:t0 + tn, :], tbl[g],
                                idx_sb[:, c0 * 8:(c0 + tn) * 8],
                                num_idxs=tn * 128, num_idxs_reg=nreg,
                                elem_size=H, queue_num=next_q(),
                                single_packet=False)
                        bufs[g] = mb
                        base_chunk[g] = ch0
                    for b in blocks:
                        nch_tot = int(plan.nchunks[b, 0] + plan.nchunks[b, 1])
                        bi = blocks.index(b)
                        pa = pagg_pool.tile([128, 128], F32, tag="pagg")
                        nc.tensor.matmul(pa[:], Is2t[b][:], hsb[b][:],
                                         start=True, stop=(nch_tot == 0))
                        done = 0
                        for g in (0, 1):
                            nch_bg = int(plan.nchunks[b, g])
                            if nch_bg == 0:
                                continue
                            loc = sum(int(plan.nchunks[bb, g])
                                      for bb in blocks[:bi])
                            mb = bufs[g]
                            for t0 in range(0, nch_bg, SB):
                                tn = min(SB, nch_bg - t0)
                                c = base_chunk[g] + loc + t0
                                sg_t = sgen_pool.tile([128, SB * 128], F16,
                                                      tag="sg")
                                dsl = dstrel_sb[:, c:c + tn]
                                nc.vector.tensor_tensor(
                                    sg_t[:, :tn * 128].rearrange(
                                        "p (c d) -> p c d", d=128),
                                    dsl.unsqueeze(2).broadcast_to(
                                        [128, tn, 128]),
                                    iota_sb[:, :tn * 128].rearrange(
                                        "p (c d) -> p c d", d=128),
                                    op=ALU.is_equal)
                                for t in range(tn):
                                    done += 1
                                    nc.tensor.matmul(
                                        pa[:],
                                        sg_t[:, t * 128:(t + 1) * 128],
                                        mb[:, loc + t0 + t, :],
                                        start=False, stop=(done == nch_tot))
                        hm = work.tile([128, 128], F32, tag="hm")
                        nc.scalar.activation(hm[:], pa[:], AF.Copy,
                                             scale=m3_sb[:, b:b + 1])
                        nc.vector.scalar_tensor_tensor(
                            h0[b][:], h0[b][:], gates[l], hm[:],
                            op0=ALU.mult, op1=ALU.add)

            # ---- lin2 ----
            for b in range(nb):
                rows = rows_of(b)
                hb = work.tile([128, 128], F16, tag="hb")
                nc.scalar.activation(hb[:], h0[b][:], AF.Relu)
                pt = pt_pool.tile([128, 128], F16, tag="pt")
                nc.tensor.transpose(pt[:], hb[:], ident[:])
                hT = work.tile([128, 128], F16, tag="hT")
                nc.scalar.activation(hT[:], pt[:], AF.Copy)
                po = ph_pool.tile([128, DO], F32, tag="ph", name="po")
                nc.tensor.matmul(po[:rows, :], hT[:, :rows], W2_sb[:, :],
                                 start=True, stop=False)
                nc.tensor.matmul(po[:rows, :], ones_sb[:, :rows], b2_sb[:],
                                 start=False, stop=True)
                ot = work.tile([128, DO], F32, tag="ot")
                nc.scalar.activation(ot[:rows, :], po[:rows, :], AF.Copy)
                nc.sync.dma_start(out_d[b * 128:b * 128 + rows, :],
                                  ot[:rows, :])

    nc.finalize()
    return nc


def make_in_maps(plan: Plan, x, W1, b1, Wr, W2, b2):
    cfg = plan.cfg
    nsh = cfg.nsh
    KI = cfg.d_in // 128
    W1m = np.ascontiguousarray(
        np.asarray(W1, np.float32).reshape(KI, 128, cfg.d_h)
        .transpose(1, 0, 2).reshape(128, KI * cfg.d_h)).astype(np.float16)
    # WrT[k, l*H+j] = Wr[l, j, k]
    WrTm = np.ascontiguousarray(
        np.asarray(Wr, np.float32).transpose(2, 0, 1)
        .reshape(128, -1)).astype(np.float16)
    iota_in = np.ascontiguousarray(
        np.tile(np.arange(128, dtype=np.float16)[None, :],
                (128, cfg.sgen_batch)))
    ident_in = np.eye(128, dtype=np.float16)
    common = dict(
        W1=W1m,
        b1r=np.ascontiguousarray(
            np.asarray(b1, np.float32).reshape(1, -1)).astype(np.float16),
        WrT=WrTm,
        W2=np.ascontiguousarray(np.asarray(W2, np.float32)).astype(np.float16),
        b2r=np.ascontiguousarray(
            np.asarray(b2, np.float32).reshape(1, -1)).astype(np.float16),
        iota_in=iota_in, ident_in=ident_in,
    )
    in_maps = []
    x = np.asarray(x, np.float32)
    for r in range(cfg.n_cores):
        pc = plan.per_core[r]
        # xT[p, k*nsh + c] = x[c, k*128 + p]
        xTm = np.ascontiguousarray(
            x[r * nsh:(r + 1) * nsh].T
            .reshape(KI, 128, nsh).transpose(1, 0, 2)
            .reshape(128, KI * nsh)).astype(np.float16)
        m = dict(common)
        m.update(
            xT=xTm, idx16=pc["idx16"], dstrel=pc["dstrel"],
            dinv_cols=pc["dinv_cols"], m3_cols=pc["m3_cols"],
            is2_cols=pc["is2_cols"],
        )
        in_maps.append(m)
    return in_maps


_cache = {}


def kernel(x, W1, b1, Wr, eps, W2, b2, edge_index, *, trace=False, cfg=None):
    cfg = cfg or Cfg()
    x = np.asarray(x)
    edge_index = np.asarray(edge_index)
    gates = [float(1.0 + math.tanh(float(e))) for e in np.asarray(eps)]

    ck = hash((edge_index.tobytes(), tuple(gates), cfg.n, cfg.n_cores,
               cfg.sg_blocks, cfg.n_queues, cfg.cmax, cfg.shared_tables,
               cfg.direct_table))
    if ck not in _cache:
        plan = preprocess(edge_index, cfg)
        nc = build_bass(plan, gates)
        _cache.clear()
        _cache[ck] = (plan, nc)
    plan, nc = _cache[ck]

    in_maps = make_in_maps(plan, x, W1, b1, Wr, W2, b2)
    try:
        res = bass_utils.run_bass_kernel_spmd(
            nc, in_maps, core_ids=list(range(cfg.n_cores)), trace=trace)
    except ModuleNotFoundError:
        # axon NTFF profiling hook unavailable in this container
        res = bass_utils.run_bass_kernel_spmd(
            nc, in_maps, core_ids=list(range(cfg.n_cores)), trace=False)
    out = np.concatenate([r["out"] for r in res.results], axis=0)
    kernel.last_results = res
    return out.astype(np.float32)


# revision 20
# speedup vs baseline: 1.0349x; 1.0349x over previous
"""EnergyGCN Trainium2 kernel: 8-core SPMD Bass/Tile implementation.

Strategy (node sharding):
  - 50000 nodes sharded contiguously across 8 cores (6250 rows, 49 blocks of
    128 per core).  Hidden state h0 kept node-major in SBUF: h0[b] =
    [128 nodes, H].
  - Per layer l: per block: hb = relu(h0) (fp16), PE-transpose -> hT [H, d],
    hr = hT.T @ WrT (PSUM, node-major), scaled by dinv on the PSUM->SBUF
    copy: hsb[b] = dinv * hr (= h~, fp16, node-major).  hsb is DMA'd into
    one of two DRAM bounce halves (blocks nbB..nb-1 -> half A first, then
    0..nbB-1 -> half B) and each half AllGather'd into a [8*half, H] fp16
    table (Shared scratchpad output).
  - Edge messages h~[col] are fetched with batched gpsimd.dma_gather (int16
    row indices into the half tables, round-robin over 4 SWDGE queues) and
    aggregated on TensorE with one-hot segment matrices generated by DVE in
    batches of 8 chunks: S[e, d] = (dstrel_e == iota_d), so PSUM accumulates
        pa[d, h] = sum_e S[e,d] * msg[e,h] + sum_d' Is2[d',d] * hsb[d',h]
    with Is2 = diag(-2*deg_d + 1).  Epilogue: the -3*dinv_d dest scale rides
    the ACT PSUM->SBUF copy (per-partition scale), then DVE:
        h0 = gate_l * h0 + (-3*dinv) * pa
    which equals gate*h0 + 6*hr - 3*ahat(hr) with self-loops folded in.
  - relu applied on read (next layer / final lin2).

Reference math:
    h = relu(x @ W1 + b1); h0 = h
    for l: hr = h @ Wr[l].T ; hn = 6hr - 3*ahat(hr)
           h0 = (1+tanh(eps[l]))*h0 + hn ; h = relu(h0)
    out = h @ W2 + b2
with ahat(y) = segment_sum(w[:,None]*y[col], row, n), w = dinv[row]*dinv[col],
self-loops appended, deg = counts of row (incl self), dinv = rsqrt(deg).
"""

import math

import numpy as np

import concourse.bacc as bacc
import concourse.bass as bass
import concourse.mybir as mybir
import concourse.tile as tile
from concourse import bass_utils

F32 = mybir.dt.float32
F16 = mybir.dt.float16
I16 = mybir.dt.int16
AF = mybir.ActivationFunctionType
ALU = mybir.AluOpType

N_NODES = 50000
D_IN, D_H, D_OUT = 256, 128, 64
N_LAYERS = 4
N_CORES = 8


class Cfg:
    def __init__(self, n=N_NODES, n_cores=N_CORES, d_in=D_IN, d_h=D_H,
                 d_out=D_OUT, n_layers=N_LAYERS, sg_blocks=6, gather_bufs=3,
                 sgen_batch=8, n_queues=4, cmax=9, shared_tables=True,
                 direct_table=False):
        self.n = n
        self.n_cores = n_cores
        self.d_in = d_in
        self.d_h = d_h
        self.d_out = d_out
        self.n_layers = n_layers
        self.sg_blocks = sg_blocks
        self.gather_bufs = gather_bufs
        self.sgen_batch = sgen_batch
        self.n_queues = n_queues
        self.cmax = cmax
        self.shared_tables = shared_tables
        self.direct_table = direct_table
        assert n % n_cores == 0
        self.nsh = n // n_cores
        self.nb = (self.nsh + 127) // 128
        # half B = blocks [0, nbB), allgathered second; half A first
        self.nbB = (self.nb + 1) // 2
        self.half_b = 128 * self.nbB
        self.ord_blocks = list(range(self.nbB, self.nb)) + list(range(self.nbB))


class Plan:
    pass


def preprocess(edge_index: np.ndarray, cfg: Cfg) -> Plan:
    n, P, nsh, nb = cfg.n, cfg.n_cores, cfg.nsh, cfg.nb
    row = edge_index[0].astype(np.int64)
    col = edge_index[1].astype(np.int64)
    deg = np.bincount(row, minlength=n).astype(np.float64) + 1.0  # + self loop
    dinv = 1.0 / np.sqrt(deg)
    m3 = (-3.0 * dinv).astype(np.float32)           # -3*dinv (dest scale)
    is2 = (-2.0 * deg + 1.0).astype(np.float32)     # identity inject diag

    # source node -> (group, row in half table);  g=0: half A (off>=half_b)
    HB = cfg.half_b
    s = col // nsh
    off = col - s * nsh
    hA_pad = 128 * (nb - cfg.nbB)
    g_of = (off < HB).astype(np.int64)              # g=0 -> half A, g=1 -> B
    rowg = np.where(off >= HB, s * hA_pad + off - HB, s * HB + off)

    ordpos = np.zeros(nb, dtype=np.int64)           # block -> position
    for i, b in enumerate(cfg.ord_blocks):
        ordpos[b] = i
    SGB = cfg.sg_blocks

    core = row // nsh
    per_core_edges = []
    counts = np.zeros((P, nb, 2), dtype=np.int64)
    for r in range(P):
        m = core == r
        rr = row[m] - r * nsh
        cc = rowg[m]
        g = g_of[m]
        blk = rr // 128
        pos = ordpos[blk]
        order = np.lexsort((rr, pos, g, pos // SGB))
        per_core_edges.append((rr[order], cc[order], g[order], blk[order]))
        np.add.at(counts[r], (blk, g), 1)

    nchunks = (counts.max(axis=0) + 127) // 128     # [nb, 2]

    # chunk layout: supergroups of SGB blocks in ord_blocks order; per sg:
    # (g0 chunks for its blocks in order) then (g1 chunks).
    slot_off = {}
    sg_entries = []
    offc = 0
    for i0 in range(0, nb, SGB):
        blocks = cfg.ord_blocks[i0:i0 + SGB]
        entries = []
        for g in (0, 1):
            c0 = offc
            for b in blocks:
                slot_off[(b, g)] = offc
                offc += int(nchunks[b, g]) * 128
            entries.append((g, (offc - c0) // 128, c0 // 128))
        sg_entries.append((blocks, entries))
    total_slots = offc
    total_chunks = total_slots // 128

    per_core = []
    for r in range(P):
        rr, cc, g, blk = per_core_edges[r]
        idx = np.zeros(total_slots, dtype=np.int16)     # pad -> row 0 (valid)
        dstrel = np.full(total_slots, -1.0, dtype=np.float16)
        for b in range(nb):
            for gg in (0, 1):
                msel = (blk == b) & (g == gg)
                k = int(msel.sum())
                if k == 0:
                    continue
                o = slot_off[(b, gg)]
                idx[o:o + k] = cc[msel]
                dstrel[o:o + k] = (rr[msel] - b * 128).astype(np.float16)
        # dma_gather index wrapping: slot i -> [i % 16, i // 16]; the Q7
        # tx/rx cores read different partition groups -> replicate to 128.
        idx16 = np.ascontiguousarray(np.tile(idx.reshape(-1, 16).T, (8, 1)))
        dstrel128 = np.ascontiguousarray(
            dstrel.reshape(total_chunks, 128).T)        # [128, nchunk] fp16

        def colmat(v):
            out = np.zeros((nb * 128,), dtype=np.float64)
            out[:nsh] = v
            return np.ascontiguousarray(
                out.reshape(nb, 128).T.astype(np.float32))

        per_core.append(dict(
            idx16=idx16,
            dstrel=dstrel128,
            dinv_cols=colmat(dinv[r * nsh:(r + 1) * nsh]),
            m3_cols=colmat(m3[r * nsh:(r + 1) * nsh]),
            is2_cols=colmat(is2[r * nsh:(r + 1) * nsh]),
        ))

    plan = Plan()
    plan.cfg = cfg
    plan.nchunks = nchunks
    plan.sg_entries = sg_entries
    plan.total_chunks = total_chunks
    plan.total_slots = total_slots
    plan.per_core = per_core
    return plan


def build_bass(plan: Plan, gates):
    cfg = plan.cfg
    nsh, nb, P = cfg.nsh, cfg.nb, cfg.n_cores
    H, DI, DO, L = cfg.d_h, cfg.d_in, cfg.d_out, cfg.n_layers
    TC = plan.total_chunks
    TS16 = plan.total_slots // 16
    SB = cfg.sgen_batch
    hA = nsh - cfg.half_b
    hA_pad = 128 * (nb - cfg.nbB)
    rowsA, rowsB = P * hA_pad, P * cfg.half_b

    nc = bacc.Bacc("TRN2", target_bir_lowering=False, debug=False,
                   num_devices=P, num_swdge_queues=cfg.n_queues)

    KI = DI // 128
    xT = nc.dram_tensor("xT", [128, KI * nsh], F16, kind="ExternalInput")
    W1 = nc.dram_tensor("W1", [128, KI * H], F16, kind="ExternalInput")
    b1r = nc.dram_tensor("b1r", [1, H], F16, kind="ExternalInput")
    WrT = nc.dram_tensor("WrT", [128, L * H], F16, kind="ExternalInput")
    W2 = nc.dram_tensor("W2", [H, DO], F16, kind="ExternalInput")
    b2r = nc.dram_tensor("b2r", [1, DO], F16, kind="ExternalInput")
    idx16_d = nc.dram_tensor("idx16", [128, TS16], I16, kind="ExternalInput")
    dstrel_d = nc.dram_tensor("dstrel", [128, TC], F16, kind="ExternalInput")
    dinv_d = nc.dram_tensor("dinv_cols", [128, nb], F32, kind="ExternalInput")
    m3_d = nc.dram_tensor("m3_cols", [128, nb], F32, kind="ExternalInput")
    is2_d = nc.dram_tensor("is2_cols", [128, nb], F32, kind="ExternalInput")
    iota_d = nc.dram_tensor("iota_in", [128, SB * 128], F16,
                            kind="ExternalInput")
    ident_d = nc.dram_tensor("ident_in", [128, 128], F16, kind="ExternalInput")
    out_d = nc.dram_tensor("out", [nsh, DO], F32, kind="ExternalOutput")

    last_rows = nsh - (nb - 1) * 128

    def rows_of(b):
        return last_rows if b == nb - 1 else 128

    addr_space = "Shared" if cfg.shared_tables else "Local"

    with tile.TileContext(nc) as tc:
        with (
            tc.tile_pool(name="const", bufs=1) as cpool,
            tc.tile_pool(name="work", bufs=4) as work,
            tc.tile_pool(name="sgen", bufs=4) as sgen_pool,
            tc.tile_pool(name="ga", bufs=cfg.gather_bufs) as ga_pool,
            tc.tile_pool(name="gb", bufs=cfg.gather_bufs) as gb_pool,
            tc.tile_pool(name="pt", bufs=2, space="PSUM") as pt_pool,
            tc.tile_pool(name="ph", bufs=2, space="PSUM") as ph_pool,
            tc.tile_pool(name="pagg", bufs=3, space="PSUM") as pagg_pool,
            tc.tile_pool(name="dram", bufs=2, space="DRAM") as dram,
        ):
            # persistent per-block state (node-major)
            h0 = [cpool.tile([128, 128], F32, tag=f"h0_{b}", name=f"h0_{b}")
                  for b in range(nb)]
            hsb = [cpool.tile([128, 128], F16, tag=f"hs_{b}", name=f"hs_{b}")
                   for b in range(nb)]
            Is2t = [cpool.tile([128, 128], F16, tag=f"i2_{b}", name=f"i2_{b}")
                    for b in range(nb)]
            for b in range(nb):
                nc.vector.memset(h0[b][:], 0.0)

            idx_sb = cpool.tile([128, TS16], I16)
            nc.sync.dma_start(idx_sb[:], idx16_d[:, :])
            dstrel_sb = cpool.tile([128, TC], F16)
            nc.sync.dma_start(dstrel_sb[:], dstrel_d[:, :])
            dinv_sb = cpool.tile([128, nb], F32)
            nc.sync.dma_start(dinv_sb[:], dinv_d[:, :])
            m3_sb = cpool.tile([128, nb], F32)
            nc.sync.dma_start(m3_sb[:], m3_d[:, :])
            is2_sb = cpool.tile([128, nb], F32)
            nc.sync.dma_start(is2_sb[:], is2_d[:, :])

            W1_sb = cpool.tile([128, KI * H], F16)
            nc.sync.dma_start(W1_sb[:], W1[:, :])
            b1_sb = cpool.tile([1, H], F16)
            nc.sync.dma_start(b1_sb[:], b1r[:, :])
            WrT_sb = cpool.tile([128, L * H], F16)
            nc.sync.dma_start(WrT_sb[:], WrT[:, :])
            W2_sb = cpool.tile([H, DO], F16)
            nc.sync.dma_start(W2_sb[:], W2[:, :])
            b2_sb = cpool.tile([1, DO], F16)
            nc.sync.dma_start(b2_sb[:], b2r[:, :])
            ones_sb = cpool.tile([1, 128], F16)
            nc.vector.memset(ones_sb[:], 1.0)
            iota_sb = cpool.tile([128, SB * 128], F16)
            nc.sync.dma_start(iota_sb[:], iota_d[:, :])
            ident = cpool.tile([128, 128], F16)
            nc.sync.dma_start(ident[:], ident_d[:, :])
            for b in range(nb):
                nc.vector.tensor_scalar(Is2t[b][:], ident[:],
                                        is2_sb[:, b:b + 1], None, op0=ALU.mult)

            # ---- lin1: h0 = relu(x @ W1 + b1) (node-major) ----
            for b in range(nb):
                rows = rows_of(b)
                ps = ph_pool.tile([128, 128], F32, tag="ph")
                for k in range(KI):
                    xs = work.tile([128, 128], F16, tag="xs")
                    nc.sync.dma_start(
                        xs[:, :rows],
                        xT[:, k * nsh + b * 128:k * nsh + b * 128 + rows])
                    nc.tensor.matmul(ps[:rows, :], xs[:, :rows],
                                     W1_sb[:, k * H:(k + 1) * H],
                                     start=(k == 0), stop=False)
                nc.tensor.matmul(ps[:rows, :], ones_sb[:, :rows], b1_sb[:],
                                 start=False, stop=True)
                nc.scalar.activation(h0[b][:rows, :], ps[:rows, :], AF.Relu)

            # ---- layers ----
            qrr = [0]

            def next_q():
                q = qrr[0]
                qrr[0] = (q + 1) % cfg.n_queues
                return q

            if cfg.direct_table:
                pidv = nc.sync.partition_id()
                offA_base = nc.sync.compute_val(pidv * (hA_pad * H))
                offB_base = nc.sync.compute_val(pidv * (cfg.half_b * H))

            barr_z = {}
            for l in range(L):
                bounceA = dram.tile([hA_pad, H], F16, tag="bA")
                bounceB = dram.tile([cfg.half_b, H], F16, tag="bB")
                tableA = dram.tile([rowsA, H], F16, tag="tA",
                                   addr_space=addr_space)
                tableB = dram.tile([rowsB, H], F16, tag="tB",
                                   addr_space=addr_space)
                if cfg.direct_table:
                    flagA = dram.tile([1, H], F16, tag="fA")
                    flagB = dram.tile([1, H], F16, tag="fB")
                    flagoutA = dram.tile([P, H], F16, tag="foA")
                    flagoutB = dram.tile([P, H], F16, tag="foB")
                taap = tableA.opt()
                tbap = tableB.opt()
                for b in cfg.ord_blocks:
                    rows = rows_of(b)
                    hb = work.tile([128, 128], F16, tag="hb")
                    nc.scalar.activation(hb[:], h0[b][:], AF.Relu)
                    pt = pt_pool.tile([128, 128], F16, tag="pt")
                    nc.tensor.transpose(pt[:], hb[:], ident[:])
                    hT = work.tile([128, 128], F16, tag="hT")
                    nc.scalar.activation(hT[:], pt[:], AF.Copy)
                    ph = ph_pool.tile([128, 128], F32, tag="ph")
                    nc.tensor.matmul(ph[:], hT[:],
                                     WrT_sb[:, l * H:(l + 1) * H],
                                     start=True, stop=True)
                    nc.scalar.activation(hsb[b][:], ph[:], AF.Copy,
                                         scale=dinv_sb[:, b:b + 1])
                    in_half_a = b >= cfg.nbB
                    r0 = (b - cfg.nbB) * 128 if in_half_a else b * 128
                    bounce = bounceA if in_half_a else bounceB
                    nc.sync.dma_start(bounce[r0:r0 + rows, :],
                                      hsb[b][:rows, :])
                    last = (b == nb - 1) if in_half_a \
                        else (b == cfg.nbB - 1)
                    if last and cfg.direct_table:
                        g = 0 if in_half_a else 1
                        bnc, tap, base, flag, flagout, nrows = (
                            (bounceA, taap, offA_base, flagA, flagoutA, hA)
                            if in_half_a else
                            (bounceB, tbap, offB_base, flagB, flagoutB,
                             cfg.half_b))
                        dst = bass.AP(tap.tensor, base,
                                      [[H, nrows], [1, H]],
                                      dep_tracking_offset=0)
                        nc.sync.dma_start(dst, bnc.opt())
                        fsb = work.tile([1, H], F16, tag="flag")
                        nc.sync.dma_start(fsb[:], tap[0:1, :])
                        nc.sync.dma_start(flag[0:1, :], fsb[:])
                        nc.gpsimd.collective_compute(
                            "AllGather", ALU.bypass,
                            replica_groups=[list(range(P))],
                            ins=[flag.opt()], outs=[flagout.opt()])
                        bsb = work.tile([1, 16], F16, tag="barr",
                                        name=f"bsb{g}_{l}")
                        nc.sync.dma_start(bsb[:], flagout[0:1, 0:16])
                        breg = nc.gpsimd.alloc_register(f"barr_{g}_{l}")
                        nc.gpsimd.reg_load(
                            breg, bsb.bitcast(mybir.dt.uint32)[0:1, 0:1])
                        barr_z[g] = nc.gpsimd.compute_val(
                            nc.gpsimd.snap(breg, donate=True) * 0)
                    elif last:
                        bnc, tbl_t = ((bounceA, tableA) if in_half_a
                                      else (bounceB, tableB))
                        nc.gpsimd.collective_compute(
                            "AllGather", ALU.bypass,
                            replica_groups=[list(range(P))],
                            ins=[bnc.opt()], outs=[tbl_t.opt()])

                tbl = {0: tableA.opt(), 1: tableB.opt()}
                gp = {0: ga_pool, 1: gb_pool}
                for si, (blocks, entries) in enumerate(plan.sg_entries):
                    bufs = {}
                    base_chunk = {}
                    for (g, nch, ch0) in entries:
                        if nch == 0:
                            continue
                        mb = gp[g].tile([128, nch, H], F16, tag=f"g{g}",
                                        name=f"mb{g}_{si}")
                        for t0 in range(0, nch, cfg.cmax):
                            tn = min(cfg.cmax, nch - t0)
                            c0 = ch0 + t0
                            nreg = (barr_z[g] + tn * 128) \
                                if cfg.direct_table else tn * 128
                            nc.gpsimd.dma_gather(
                                mb[:, t0:t0 + tn, :], tbl[g],
                                idx_sb[:, c0 * 8:(c0 + tn) * 8],
                                num_idxs=tn * 128, num_idxs_reg=nreg,
                                elem_size=H, queue_num=next_q(),
                                single_packet=False)
                        bufs[g] = mb
                        base_chunk[g] = ch0
                    for b in blocks:
                        nch_tot = int(plan.nchunks[b, 0] + plan.nchunks[b, 1])
                        bi = blocks.index(b)
                        pa = pagg_pool.tile([128, 128], F32, tag="pagg")
                        nc.tensor.matmul(pa[:], Is2t[b][:], hsb[b][:],
                                         start=True, stop=(nch_tot == 0))
                        done = 0
                        for g in (0, 1):
                            nch_bg = int(plan.nchunks[b, g])
                            if nch_bg == 0:
                                continue
                            loc = sum(int(plan.nchunks[bb, g])
                                      for bb in blocks[:bi])
                            mb = bufs[g]
                            for t0 in range(0, nch_bg, SB):
                                tn = min(SB, nch_bg - t0)
                                c = base_chunk[g] + loc + t0
                                sg_t = sgen_pool.tile([128, SB * 128], F16,
                                                      tag="sg")
                                dsl = dstrel_sb[:, c:c + tn]
                                nc.vector.tensor_tensor(
                                    sg_t[:, :tn * 128].rearrange(
                                        "p (c d) -> p c d", d=128),
                                    dsl.unsqueeze(2).broadcast_to(
                                        [128, tn, 128]),
                                    iota_sb[:, :tn * 128].rearrange(
                                        "p (c d) -> p c d", d=128),
                                    op=ALU.is_equal)
                                for t in range(tn):
                                    done += 1
                                    nc.tensor.matmul(
                                        pa[:],
                                        sg_t[:, t * 128:(t + 1) * 128],
                                        mb[:, loc + t0 + t, :],
                                        start=False, stop=(done == nch_tot))
                        hm = work.tile([128, 128], F32, tag="hm")
                        nc.scalar.activation(hm[:], pa[:], AF.Copy,
                                             scale=m3_sb[:, b:b + 1])
                        nc.vector.scalar_tensor_tensor(
                            h0[b][:], h0[b][:], gates[l], hm[:],
                            op0=ALU.mult, op1=ALU.add)

            # ---- lin2 ----
            for b in range(nb):
                rows = rows_of(b)
                hb = work.tile([128, 128], F16, tag="hb")
                nc.scalar.activation(hb[:], h0[b][:], AF.Relu)
                pt = pt_pool.tile([128, 128], F16, tag="pt")
                nc.tensor.transpose(pt[:], hb[:], ident[:])
                hT = work.tile([128, 128], F16, tag="hT")
                nc.scalar.activation(hT[:], pt[:], AF.Copy)
                po = ph_pool.tile([128, DO], F32, tag="ph", name="po")
                nc.tensor.matmul(po[:rows, :], hT[:, :rows], W2_sb[:, :],
                                 start=True, stop=False)
                nc.tensor.matmul(po[:rows, :], ones_sb[:, :rows], b2_sb[:],
                                 start=False, stop=True)
                ot = work.tile([128, DO], F32, tag="ot")
                nc.scalar.activation(ot[:rows, :], po[:rows, :], AF.Copy)
                nc.sync.dma_start(out_d[b * 128:b * 128 + rows, :],
                                  ot[:rows, :])

    nc.finalize()
    return nc


def make_in_maps(plan: Plan, x, W1, b1, Wr, W2, b2):
    cfg = plan.cfg
    nsh = cfg.nsh
    KI = cfg.d_in // 128
    W1m = np.ascontiguousarray(
        np.asarray(W1, np.float32).reshape(KI, 128, cfg.d_h)
        .transpose(1, 0, 2).reshape(128, KI * cfg.d_h)).astype(np.float16)
    # WrT[k, l*H+j] = Wr[l, j, k]
    WrTm = np.ascontiguousarray(
        np.asarray(Wr, np.float32).transpose(2, 0, 1)
        .reshape(128, -1)).astype(np.float16)
    iota_in = np.ascontiguousarray(
        np.tile(np.arange(128, dtype=np.float16)[None, :],
                (128, cfg.sgen_batch)))
    ident_in = np.eye(128, dtype=np.float16)
    common = dict(
        W1=W1m,
        b1r=np.ascontiguousarray(
            np.asarray(b1, np.float32).reshape(1, -1)).astype(np.float16),
        WrT=WrTm,
        W2=np.ascontiguousarray(np.asarray(W2, np.float32)).astype(np.float16),
        b2r=np.ascontiguousarray(
            np.asarray(b2, np.float32).reshape(1, -1)).astype(np.float16),
        iota_in=iota_in, ident_in=ident_in,
    )
    in_maps = []
    x = np.asarray(x, np.float32)
    for r in range(cfg.n_cores):
        pc = plan.per_core[r]
        # xT[p, k*nsh + c] = x[c, k*128 + p]
        xTm = np.ascontiguousarray(
            x[r * nsh:(r + 1) * nsh].T
            .reshape(KI, 128, nsh).transpose(1, 0, 2)
            .reshape(128, KI * nsh)).astype(np.float16)
        m = dict(common)
        m.update(
            xT=xTm, idx16=pc["idx16"], dstrel=pc["dstrel"],
            dinv_cols=pc["dinv_cols"], m3_cols=pc["m3_cols"],
            is2_cols=pc["is2_cols"],
        )
        in_maps.append(m)
    return in_maps


_cache = {}


def kernel(x, W1, b1, Wr, eps, W2, b2, edge_index, *, trace=False, cfg=None):
    cfg = cfg or Cfg()
    x = np.asarray(x)
    edge_index = np.asarray(edge_index)
    gates = [float(1.0 + math.tanh(float(e))) for e in np.asarray(eps)]

    ck = hash((edge_index.tobytes(), tuple(gates), cfg.n, cfg.n_cores,
               cfg.sg_blocks, cfg.n_queues, cfg.cmax, cfg.shared_tables,
               cfg.direct_table))
    if ck not in _cache:
        plan = preprocess(edge_index, cfg)
        nc = build_bass(plan, gates)
        _cache.clear()
        _cache[ck] = (plan, nc)
    plan, nc = _cache[ck]

    in_maps = make_in_maps(plan, x, W1, b1, Wr, W2, b2)
    try:
        res = bass_utils.run_bass_kernel_spmd(
            nc, in_maps, core_ids=list(range(cfg.n_cores)), trace=trace)
    except ModuleNotFoundError:
        # axon NTFF profiling hook unavailable in this container
        res = bass_utils.run_bass_kernel_spmd(
            nc, in_maps, core_ids=list(range(cfg.n_cores)), trace=False)
    out = np.concatenate([r["out"] for r in res.results], axis=0)
    kernel.last_results = res
    return out.astype(np.float32)
